# revision 55
# baseline (speedup 1.0000x reference)
"""AttentionBlock (GroupNorm + MHA + proj + residual) on 8 Trainium2 cores.

Sharding: data-parallel over batch (b=8, one sample per NeuronCore).
Per-core kernel computes the full block for one sample entirely on-chip:

  x [512, 1024] (bf16) -> GroupNorm(32 groups) -> qkv (bf16 matmuls)
    -> per-head QK^T (K=64, two heads row-tiled into concurrent PE halves)
    -> exp on ScalarE (softmax denominator via an extra ones column in the
       AV matmul's stationary operand)
    -> AV (K=128) -> normalize -> proj + bias + residual -> out [512, 1024]

v2 changes vs the first working kernel:
  - input is bf16-only (residual from the bf16 copy); input DMA halved and
    spread over 4 engine queues so chunks land in parallel
  - GroupNorm rstd via a DVE bit-trick + 2 Newton steps: no Ln/Exp round
    trip on ScalarE, so the whole kernel uses ONE activation table set
    (exp_and_others: copy/identity/exp) -> one 1.3us table load, not three
  - full-chunk stats passes (Sx on ACT with accum, Sxx on DVE with accum)
  - kc-major emission for pair-0 qkv so it pipelines behind the per-chunk
    affine; q/k bias adds split between ACT and DVE
  - softmax normalize: reciprocal straight from the PSUM denominator row,
    gpsimd broadcast, one DVE multiply (numerator read from PSUM) -- no
    ScalarE involvement in the steady state (ACT runs exp back-to-back)
  - pair-3 runs two of its own AV sweeps inside its S loop; the remaining
    two + proj make a short tail
"""
import sys

sys.path.insert(0, "/opt/trn_rl_repo")

import numpy as np

import concourse.bacc as bacc
import concourse.mybir as mybir
from concourse.bass_utils import run_bass_kernel_spmd
from concourse.tile import TileContext

AF = mybir.ActivationFunctionType
OP = mybir.AluOpType
F32 = mybir.dt.float32
U32 = mybir.dt.uint32
BF16 = mybir.dt.bfloat16

B, C, HH, WW = 8, 512, 32, 32
L = HH * WW          # 1024
H = 8                # heads
HD = C // H          # 64
G = 32               # groups
GSZ = C // G         # 16 channels per group
EPS = 1e-5
N_CORES = 8
EXP_BUFS = 32
MAGIC = 0x5F3759DF

_CACHE = {}


def _build_module():
    if "nc" in _CACHE:
        return _CACHE["nc"]
    nc = bacc.Bacc("TRN2", target_bir_lowering=False, debug=False)

    xb_d = nc.dram_tensor("xb", [C, L], BF16, kind="ExternalInput")
    wqk_d = nc.dram_tensor("wqk", [C, 2 * C], BF16, kind="ExternalInput")
    bqk_d = nc.dram_tensor("bqk", [128, 8], F32, kind="ExternalInput")
    wv_d = nc.dram_tensor("wv", [C, C], BF16, kind="ExternalInput")
    bvb_d = nc.dram_tensor("bvb", [1, C], F32, kind="ExternalInput")
    wp_d = nc.dram_tensor("wp", [C, C], BF16, kind="ExternalInput")
    pb_d = nc.dram_tensor("pb", [128, 4], F32, kind="ExternalInput")
    gfw_d = nc.dram_tensor("gfw", [128, 128], F32, kind="ExternalInput")
    gbw_d = nc.dram_tensor("gbw", [G, C], F32, kind="ExternalInput")
    ones8_d = nc.dram_tensor("ones8", [128, 8], BF16, kind="ExternalInput")
    out_d = nc.dram_tensor("out", [C, L], F32, kind="ExternalOutput")
    dbg_d = nc.dram_tensor("dbg", [C, L], F32, kind="ExternalOutput") if _CACHE.get("debug") else None

    with TileContext(nc) as tc:
        with tc.tile_pool(name="persist", bufs=1) as per, \
             tc.tile_pool(name="expp", bufs=EXP_BUFS) as expp, \
             tc.tile_pool(name="outp", bufs=3) as outp, \
             tc.tile_pool(name="small", bufs=4) as smallp, \
             tc.tile_pool(name="sps", bufs=3, space="PSUM") as spp, \
             tc.tile_pool(name="ap4", bufs=2, space="PSUM") as ap4:

            # ---------- persistent tiles ----------
            xbt = [per.tile([128, L], BF16, tag=f"xb{j}", name=f"xb{j}") for j in range(4)]
            xn = [per.tile([128, L], BF16, tag=f"xn{j}", name=f"xn{j}") for j in range(4)]
            a_t = [per.tile([128, L], BF16, tag=f"a{j}", name=f"a{j}") for j in range(4)]
            qp = [per.tile([128, L], BF16, tag=f"qp{j}", name=f"qp{j}") for j in range(4)]
            kp = [per.tile([128, L], BF16, tag=f"kp{j}", name=f"kp{j}") for j in range(4)]
            vt = [per.tile([128, H * (HD + 1)], BF16, tag=f"vt{j}", name=f"vt{j}") for j in range(8)]
            wqk = [per.tile([128, 2 * C], BF16, tag=f"wqk{k}", name=f"wqk{k}") for k in range(4)]
            wv = [per.tile([128, C], BF16, tag=f"wv{k}", name=f"wv{k}") for k in range(4)]
            wp = [per.tile([128, C], BF16, tag=f"wp{k}", name=f"wp{k}") for k in range(4)]
            gfw_t = per.tile([128, 128], F32, tag="gfw", name="gfw")
            gbw_t = per.tile([G, C], F32, tag="gbw", name="gbw")
            bqk_t = per.tile([128, 8], F32, tag="bqk", name="bqk")
            pb_t = per.tile([128, 4], F32, tag="pb", name="pb")
            ones8_t = per.tile([128, 8], BF16, tag="ones8", name="ones8")
            bvr_t = per.tile([1, C], F32, tag="bvr", name="bvr")
            bvb_t = per.tile([128, C], F32, tag="bvb", name="bvb")

            # ---------- input DMAs, spread across engine queues ----------
            # (only SP/Activation/GpSimd can issue DMAs); xb chunks go in
            # halves across queues so stats can start ~2us earlier
            half_eng = {(0, 0): nc.sync, (0, 1): nc.scalar,
                        (1, 0): nc.sync, (1, 1): nc.gpsimd,
                        (2, 0): nc.sync, (2, 1): nc.gpsimd,
                        (3, 0): nc.sync, (3, 1): nc.gpsimd}
            for j in range(2):
                for hf in range(2):
                    half_eng[(j, hf)].dma_start(
                        out=xbt[j][:, 512 * hf:512 * hf + 512],
                        in_=xb_d[128 * j:128 * j + 128, 512 * hf:512 * hf + 512])
            nc.sync.dma_start(out=gfw_t[:, :], in_=gfw_d[:, :])
            for j in range(2, 4):
                for hf in range(2):
                    half_eng[(j, hf)].dma_start(
                        out=xbt[j][:, 512 * hf:512 * hf + 512],
                        in_=xb_d[128 * j:128 * j + 128, 512 * hf:512 * hf + 512])
            nc.sync.dma_start(out=gbw_t[:, :], in_=gbw_d[:, :])
            nc.gpsimd.dma_start(out=bqk_t[:, :], in_=bqk_d[:, :])
            nc.gpsimd.dma_start(out=ones8_t[:, :], in_=ones8_d[:, :])
            nc.gpsimd.dma_start(out=bvr_t[:, :], in_=bvb_d[:, :])
            for k in range(4):
                nc.sync.dma_start(out=wqk[k][:, :],
                                  in_=wqk_d[128 * k:128 * k + 128, :])
            for k in range(4):
                nc.gpsimd.dma_start(out=wv[k][:, :],
                                    in_=wv_d[128 * k:128 * k + 128, :])
            for k in range(4):
                nc.sync.dma_start(out=wp[k][:, :], in_=wp_d[128 * k:128 * k + 128, :])
            nc.sync.dma_start(out=pb_t[:, :], in_=pb_d[:, :])
            nc.gpsimd.partition_broadcast(bvb_t[:, :], bvr_t[:, :], channels=128)
            ones1 = per.tile([1, 128], F32, tag="ones1", name="ones1")
            nc.vector.memset(ones1[:, :], 1.0)

            def warm(n):
                wup = ap4.tile([128, 128], F32, tag="acc", name="acc")
                for _ in range(n):
                    nc.tensor.matmul(wup[:, :], gfw_t[:, :], gfw_t[:, :],
                                     start=True, stop=True)

            warm(10)

            # ---------- GroupNorm stats (per chunk) ----------
            # stats[j][:, 0] = sum_l x, stats[j][:, 1] = sum_l x^2
            stats = [per.tile([128, 2], F32, tag=f"st{j}", name=f"st{j}") for j in range(4)]
            gss = per.tile([G, 2], F32, tag="gss", name="gss")
            gstp = ap4.tile([G, 2], F32, tag="acc", name="acc")
            for j in range(4):
                nc.scalar.activation(out=a_t[j][:, :], in_=xbt[j][:, :],
                                     func=AF.Copy,
                                     accum_out=stats[j][:, 0:1])
                nc.vector.scalar_tensor_tensor(out=xn[j][:, :],
                                               in0=xbt[j][:, :],
                                               scalar=1.0, in1=xbt[j][:, :],
                                               op0=OP.mult, op1=OP.mult,
                                               accum_out=stats[j][:, 1:2])
                nc.tensor.matmul(gstp[:, :], gfw_t[:, 32 * j:32 * j + 32],
                                 stats[j][:, :], start=(j == 0), stop=(j == 3))
            nc.vector.tensor_copy(gss[:, :], gstp[:, :])
            # keep HAM hot through the group-chain + affine windows with
            # matmuls that DEPEND on gss: the scheduler cannot hoist them
            # ahead of the stats->gst chain, so no coalesced wait ever
            # includes them ahead of real work
            # gw depends on gss (ScalarE is idle here), so these N=512 fill
            # matmuls cannot be hoisted ahead of the stats chain; they keep
            # the PE array dense through the group-chain window
            gw = per.tile([G, 512], F32, tag="gw", name="gw")
            nc.scalar.activation(out=gw[:, :], in_=gbw_t[:, :],
                                 func=AF.Copy, scale=gss[:, 0:1])
            wup2 = ap4.tile([128, 512], F32, tag="acc", name="acc")
            for _ in range(8):
                nc.tensor.matmul(wup2[:, :], gbw_t[:, 0:128], gw[:, :],
                                 start=True, stop=True)

            # ---------- group chain: mean/var -> rstd via bit-trick ----------
            mean = per.tile([G, 1], F32, tag="mean", name="mean")
            nmean = per.tile([G, 1], F32, tag="nmean", name="nmean")
            e2e = per.tile([G, 1], F32, tag="e2e", name="e2e")
            veps = per.tile([G, 1], F32, tag="veps", name="veps")
            vh = per.tile([G, 1], F32, tag="vh", name="vh")
            magic = per.tile([G, 1], U32, tag="magic", name="magic")
            c15 = per.tile([G, 1], F32, tag="c15", name="c15")
            ush = per.tile([G, 1], U32, tag="ush", name="ush")
            y = per.tile([G, 1], F32, tag="y0", name="y0")
            t1 = per.tile([G, 1], F32, tag="t1", name="t1")
            u1 = per.tile([G, 1], F32, tag="u1", name="u1")
            y1 = per.tile([G, 1], F32, tag="y1", name="y1")
            t2 = per.tile([G, 1], F32, tag="t2", name="t2")
            u2 = per.tile([G, 1], F32, tag="u2", name="u2")
            gsb = per.tile([G, 2], F32, tag="gsb", name="gsb")

            nc.vector.memset(magic[:, :], MAGIC)
            nc.vector.memset(c15[:, :], 1.5)
            inv_n = 1.0 / (GSZ * L)
            nc.vector.tensor_scalar(out=mean[:, :], in0=gss[:, 0:1],
                                    scalar1=inv_n, scalar2=None, op0=OP.mult)
            nc.vector.tensor_scalar(out=e2e[:, :], in0=gss[:, 1:2],
                                    scalar1=inv_n, scalar2=EPS,
                                    op0=OP.mult, op1=OP.add)
            nc.vector.tensor_scalar(out=nmean[:, :], in0=mean[:, :],
                                    scalar1=-1.0, scalar2=None, op0=OP.mult)
            # veps = e2e - mean^2 = (mean * nmean) + e2e
            nc.vector.scalar_tensor_tensor(out=veps[:, :], in0=mean[:, :],
                                           scalar=nmean[:, 0:1], in1=e2e[:, :],
                                           op0=OP.mult, op1=OP.add)
            # rsqrt seed: y = bitcast(MAGIC - (bitcast(veps) >> 1))
            nc.vector.tensor_scalar(out=ush[:, :], in0=veps[:, :].bitcast(U32),
                                    scalar1=1, scalar2=None,
                                    op0=OP.logical_shift_right)
            nc.vector.scalar_tensor_tensor(out=y[:, :].bitcast(U32),
                                           in0=magic[:, :], scalar=0,
                                           in1=ush[:, :],
                                           op0=OP.bypass, op1=OP.subtract)
            nc.vector.tensor_scalar(out=vh[:, :], in0=veps[:, :],
                                    scalar1=0.5, scalar2=None, op0=OP.mult)
            # Newton 1: y1 = -(y * (1.5 - 0.5 v y^2))
            nc.vector.tensor_tensor(out=t1[:, :], in0=y[:, :], in1=y[:, :],
                                    op=OP.mult)
            nc.vector.scalar_tensor_tensor(out=u1[:, :], in0=t1[:, :],
                                           scalar=vh[:, 0:1], in1=c15[:, :],
                                           op0=OP.mult, op1=OP.subtract)
            nc.vector.tensor_tensor(out=y1[:, :], in0=y[:, :], in1=u1[:, :],
                                    op=OP.mult)
            # Newton 2: y2 = y1m * -(1.5 - 0.5 v y1^2)  (signs cancel)
            nc.vector.tensor_tensor(out=t2[:, :], in0=y1[:, :], in1=y1[:, :],
                                    op=OP.mult)
            nc.vector.scalar_tensor_tensor(out=u2[:, :], in0=t2[:, :],
                                           scalar=vh[:, 0:1], in1=c15[:, :],
                                           op0=OP.mult, op1=OP.subtract)
            nc.vector.tensor_tensor(out=gsb[:, 0:1], in0=y1[:, :], in1=u2[:, :],
                                    op=OP.mult)
            nc.vector.tensor_tensor(out=gsb[:, 1:2], in0=nmean[:, :],
                                    in1=gsb[:, 0:1], op=OP.mult)

            # ---------- per-channel affine coefficients + apply ----------
            cb = [per.tile([128, 2], F32, tag=f"cb{j}", name=f"cb{j}") for j in range(4)]
            for j in range(4):
                cbp = ap4.tile([128, 2], F32, tag="acc", name="acc")
                nc.tensor.matmul(cbp[:, :], gbw_t[:, 128 * j:128 * j + 128],
                                 gsb[:, :], start=True, stop=True)
                nc.vector.tensor_copy(cb[j][:, :], cbp[:, :])
                if j % 2 == 0:
                    nc.scalar.activation(out=xn[j][:, :], in_=xbt[j][:, :],
                                         func=AF.Identity,
                                         bias=cb[j][:, 1:2], scale=cb[j][:, 0:1])
                else:
                    nc.vector.tensor_scalar(out=xn[j][:, :], in0=xbt[j][:, :],
                                            scalar1=cb[j][:, 0:1],
                                            scalar2=cb[j][:, 1:2],
                                            op0=OP.mult, op1=OP.add)

            # ---------- qkv pair 0: kc-major so it pipelines behind affine ----
            # (4 concurrently-open accumulators: 2 from ap4, 2 from the
            # not-yet-used S pool)
            pq0 = {}
            for kc in range(4):
                for gi, (m, n2) in enumerate([(0, 0), (0, 1), (4, 0), (4, 1)]):
                    if kc == 0:
                        pool = ap4 if gi < 2 else spp
                        pq0[gi] = pool.tile([128, 512], F32, tag="acc" if gi < 2 else "sps",
                                            name="acc" if gi < 2 else "sps")
                    nc.tensor.matmul(pq0[gi][:, :],
                                     wqk[kc][:, 128 * m:128 * m + 128],
                                     xn[kc][:, 512 * n2:512 * n2 + 512],
                                     start=(kc == 0), stop=(kc == 3))
            # bias adds: split ACT/DVE; n2=0 halves first so S(sc=0) can start
            for gi, (m, n2) in enumerate([(4, 0), (0, 0), (0, 1), (4, 1)]):
                dest = qp[0] if m < 4 else kp[0]
                dsl = dest[:, 512 * n2:512 * n2 + 512]
                if gi % 2 == 0:
                    nc.scalar.activation(out=dsl, in_=pq0[[2, 0, 1, 3][gi]][:, :],
                                         func=AF.Identity,
                                         bias=bqk_t[:, m:m + 1])
                else:
                    nc.vector.tensor_scalar(out=dsl, in0=pq0[[2, 0, 1, 3][gi]][:, :],
                                            scalar1=bqk_t[:, m:m + 1],
                                            scalar2=None, op0=OP.add)

            # ---------- helpers ----------
            class QkvStream:
                """qkv chunks for pairs 1-3 as an emit-on-demand stream.
                n2=0 halves for both q and k come first: the next pair's
                first S chunks only need them (k's n2=1 half is first read
                at s-chunk 4)."""
                def __init__(self, ms):
                    self.jobs = [(m, n2) for n2 in range(2) for m in ms]
                    self.i = 0
                    self.pq = None

                def emit(self, k):
                    for _ in range(k):
                        if self.i >= 4 * len(self.jobs):
                            return
                        job, kc = divmod(self.i, 4)
                        m, n2 = self.jobs[job]
                        if kc == 0:
                            self.pq = ap4.tile([128, 512], F32, tag="acc",
                                               name="acc")
                        nc.tensor.matmul(self.pq[:, :],
                                         wqk[kc][:, 128 * m:128 * m + 128],
                                         xn[kc][:, 512 * n2:512 * n2 + 512],
                                         start=(kc == 0), stop=(kc == 3))
                        if kc == 3:
                            dest = qp[m % 4] if m < 4 else kp[m - 4]
                            nc.vector.tensor_scalar(
                                out=dest[:, 512 * n2:512 * n2 + 512],
                                in0=self.pq[:, :],
                                scalar1=bqk_t[:, m:m + 1], scalar2=None,
                                op0=OP.add)
                        self.i += 1

            def vt_chunk(sc):
                """v^T for s-chunk sc, all heads: [128 s, 8*(64+1)] with a
                ones column per head (softmax denominator accumulator)."""
                pv = ap4.tile([128, 512], F32, tag="acc", name="acc")
                for kc in range(4):
                    nc.tensor.matmul(pv[:, :],
                                     xn[kc][:, 128 * sc:128 * sc + 128],
                                     wv[kc][:, :], start=(kc == 0), stop=(kc == 3))
                v3 = vt[sc][:, :].rearrange("p (h e) -> p h e", e=HD + 1)
                nc.vector.tensor_copy(vt[sc][:, HD::HD + 1], ones8_t[:, :])
                nc.vector.tensor_tensor(
                    out=v3[:, :, 0:HD],
                    in0=pv[:, :].rearrange("p (h e) -> p h e", e=HD),
                    in1=bvb_t[:, :].rearrange("p (h e) -> p h e", e=HD),
                    op=OP.add)

            def norm_head(p, e, n2, pa, act_copy=False):
                """softmax-normalize one AV accumulator into a_t.  The PSUM
                accumulator is drained immediately (denominator row + raw
                numerator) so its slot recycles fast; the normalization then
                runs SBUF-side in place.  act_copy routes the drain copies
                to ScalarE (for tail norms, when it has no exp work left)."""
                base = 64 * e
                asl = a_t[p][base:base + 64, 512 * n2:512 * n2 + 512]
                dsb = smallp.tile([1, 512], F32, tag="dsb", name="dsb")
                if act_copy:
                    nc.scalar.copy(dsb[:, :], pa[HD:HD + 1, :])
                    nc.scalar.copy(asl, pa[0:HD, :])
                else:
                    nc.vector.tensor_copy(dsb[:, :], pa[HD:HD + 1, :])
                    nc.vector.tensor_copy(asl, pa[0:HD, :])
                rcp = smallp.tile([1, 512], F32, tag="rcp", name="rcp")
                nc.vector.reciprocal_approx_fast(out=rcp[:, :],
                                                 in_=dsb[:, :])
                db = smallp.tile([128, 512], F32, tag="db", name="db")
                nc.gpsimd.partition_broadcast(db[:, :], rcp[:, :],
                                              channels=128)
                nc.vector.tensor_tensor(out=asl, in0=asl,
                                        in1=db[base:base + 64, :],
                                        op=OP.mult)

            class AvStream:
                """AV accumulation sweeps as an emit-on-demand stream
                (8 matmuls per sweep; norm emitted when a sweep closes).
                lockstep=True advances all sweeps one s-chunk at a time so a
                pair's own sweeps can ride its S loop, gated only on the
                exp tiles already produced."""
                def __init__(self, pe, sweeps, lockstep=False, act_copy=False,
                             pool=None):
                    self.p, self.est = pe
                    self.sweeps = sweeps
                    self.lockstep = lockstep
                    self.act_copy = act_copy
                    self.pool = pool or ap4
                    self.i = 0
                    self.pa = [None] * len(self.sweeps)

                def emit(self, k):
                    for _ in range(k):
                        if self.i >= 8 * len(self.sweeps):
                            return
                        if self.lockstep:
                            sweep = self.i % len(self.sweeps)
                            sc = self.i // len(self.sweeps)
                        else:
                            sweep, sc = divmod(self.i, 8)
                        e, n2 = self.sweeps[sweep]
                        h = 2 * self.p + e
                        if sc == 0:
                            self.pa[sweep] = self.pool.tile(
                                [HD + 1, 512], F32,
                                tag="acc" if self.pool is ap4 else "sps",
                                name="acc" if self.pool is ap4 else "sps")
                        nc.tensor.matmul(
                            self.pa[sweep][:, :], vt[sc][:, 65 * h:65 * h + 65],
                            self.est[e][sc][:, 512 * n2:512 * n2 + 512],
                            start=(sc == 0), stop=(sc == 7))
                        if sc == 7:
                            norm_head(self.p, e, n2, self.pa[sweep],
                                      act_copy=self.act_copy)
                        self.i += 1

            def s_mm(p, e, sc, est):
                """one head's S^T chunk + its exp"""
                base = 64 * e
                ps_s = spp.tile([128, L], F32, tag="sps", name="sps")
                for n2 in range(2):
                    nc.tensor.matmul(
                        ps_s[:, 512 * n2:512 * n2 + 512],
                        kp[p][base:base + 64, 128 * sc:128 * sc + 128],
                        qp[p][base:base + 64, 512 * n2:512 * n2 + 512],
                        start=True, stop=True, tile_position=(base, 0))
                es = expp.tile([128, L], BF16, tag="expS", name="expS")
                nc.scalar.activation(out=es[:, :], in_=ps_s[:, :], func=AF.Exp)
                est[e][sc] = es

            def attn_S(p, prev=None, qkv=None, own=None, stream_vt=False):
                """S^T + exp for pair p; the previous pair's AV sweeps, pair
                p+1's qkv, and (for p=3) the pair's own first sweep ride
                along ahead of the S matmuls.  The e=0/e=1 S matmul pairs
                stay adjacent so their disjoint row-groups execute
                concurrently on the PE; with 3 S-PSUM slots they are gated
                by the exp three allocations back, which has always
                drained."""
                est = [[None] * 8, [None] * 8]
                if own is not None:
                    own.est = est
                for sc in range(8):
                    s_mm(p, 0, sc, est)
                    s_mm(p, 1, sc, est)
                    if own is not None and sc >= 1:
                        own.emit(1)
                    if prev is not None:
                        prev.emit(4)
                    if stream_vt:
                        vt_chunk(sc)
                    if qkv is not None:
                        qkv.emit(2)
                return est

            # ---------- emission schedule ----------
            prev = None
            own3 = None
            for p in range(4):
                qs = QkvStream([p + 1, p + 5]) if p + 1 < 4 else None
                own3 = AvStream((p, None), sweeps=[(0, 0)],
                                lockstep=True) if p == 3 else None
                est_cur = attn_S(p, prev, qs, own=own3, stream_vt=(p == 0))
                if prev is not None:
                    prev.emit(32)  # drain previous pair's sweeps
                if qs is not None:
                    qs.emit(16)    # drain qkv remainder
                prev = AvStream((p, est_cur),
                                sweeps=[(0, 0), (1, 0), (0, 1), (1, 1)])
            own3.emit(16)          # drain pair 3's first sweep + its norm
            # tail sweeps use the now-free S PSUM slots so all three can be
            # in flight at once instead of serializing through 2 slots
            av_rest = AvStream((3, prev.est),
                               sweeps=[(1, 0), (1, 1), (0, 1)], act_copy=True,
                               pool=spp)

            class ProjStream:
                """proj groups (m, n2): 4 accumulating matmuls then fused
                bias+residual and the output DMA.  body(m) emits the three
                cc<3 matmuls (gated only on pairs 0-2, long since ready);
                close(m) emits the cc=3 matmul (gated on pair 3's norms) and
                the drain.  The drain alternates DVE and ScalarE."""
                def __init__(self, n2):
                    self.n2 = n2
                    self.po = {}

                def body(self, m):
                    self.po[m] = spp.tile([128, 512], F32, tag="sps",
                                          name="sps")
                    for cc in range(3):
                        nc.tensor.matmul(self.po[m][:, :],
                                         wp[cc][:, 128 * m:128 * m + 128],
                                         a_t[cc][:, 512 * self.n2:512 * self.n2 + 512],
                                         start=(cc == 0), stop=False)

                def close(self, m, act=False):
                    n2 = self.n2
                    nc.tensor.matmul(self.po[m][:, :],
                                     wp[3][:, 128 * m:128 * m + 128],
                                     a_t[3][:, 512 * n2:512 * n2 + 512],
                                     start=False, stop=True)
                    ob = outp.tile([128, 512], F32, tag="ob", name="ob")
                    osl = out_d[128 * m:128 * m + 128, 512 * n2:512 * n2 + 512]
                    # ScalarE drains psum+bias (fast slot release, off the
                    # congested DVE); DVE then adds the residual in place
                    nc.scalar.activation(out=ob[:, :], in_=self.po[m][:, :],
                                         func=AF.Identity,
                                         bias=pb_t[:, m:m + 1])
                    nc.vector.tensor_tensor(
                        out=ob[:, :], in0=ob[:, :],
                        in1=xbt[m][:, 512 * n2:512 * n2 + 512], op=OP.add)
                    nc.sync.dma_start(out=osl, in_=ob[:, :])

            if dbg_d is not None:
                dbg_src = _CACHE.get("debug_src", "xn")
                src = {"xn": xn, "qp": qp, "kp": kp, "at": a_t}[dbg_src]
                for j in range(4):
                    dcp = outp.tile([128, L], F32, tag="dbg", name="dbg")
                    nc.vector.tensor_copy(dcp[:, :], src[j][:, :])
                    nc.sync.dma_start(out=dbg_d[128 * j:128 * j + 128, :],
                                      in_=dcp[:, :])

            # tail: pair 3's remaining sweeps interleaved with proj bodies
            # (only pairs 0-2 needed) so the PE never idles; group closes
            # follow as pair 3's norms land.  Allocation order matches the
            # release order of the 3 S-pool slots to avoid FIFO deadlock.
            pr0, pr1 = ProjStream(0), ProjStream(1)
            av_rest.emit(8)    # (1,0)
            av_rest.emit(8)    # (1,1)
            pr0.body(0)
            av_rest.emit(8)    # (0,1)
            pr0.body(1)
            pr0.close(0)
            pr0.body(2)
            pr0.close(1)
            pr0.body(3)
            pr0.close(2)
            pr0.close(3)
            pr1.body(0)
            pr1.body(1)
            pr1.body(2)
            pr1.close(0)
            pr1.body(3)
            pr1.close(1)
            pr1.close(2)
            pr1.close(3)

    nc.compile()
    _CACHE["nc"] = nc
    return nc


def _prep_constants(norm_w, norm_b, qkv_w, qkv_b, proj_w, proj_b):
    norm_w = np.asarray(norm_w, np.float64)
    norm_b = np.asarray(norm_b, np.float64)
    qkv_w = np.asarray(qkv_w, np.float64)
    qkv_b = np.asarray(qkv_b, np.float64)
    proj_w = np.asarray(proj_w, np.float64)
    proj_b = np.asarray(proj_b, np.float64)

    idx = np.arange(HD)
    q_idx = np.concatenate([h * 3 * HD + idx for h in range(H)])
    k_idx = q_idx + HD
    v_idx = q_idx + 2 * HD

    # fold norm affine: qkv = W @ (gn*nw + nb) + b = (W*nw) @ gn + (W@nb + b)
    Wf = qkv_w * norm_w[None, :]
    bf = qkv_b + qkv_w @ norm_b
    s2 = 1.0 / np.sqrt(HD)  # both q*scale and k*scale -> fold s^2 into q
    Wq, bq = Wf[q_idx] * s2, bf[q_idx] * s2
    Wk, bk = Wf[k_idx], bf[k_idx]
    Wv, bv = Wf[v_idx], bf[v_idx]

    wqk = np.concatenate([Wq.T, Wk.T], axis=1)                  # [512, 1024]
    bqk = np.concatenate([bq, bk]).reshape(8, 128).T            # [128, 8]
    wv = np.ascontiguousarray(Wv.T)                             # [512, 512]
    wp = np.ascontiguousarray(proj_w.T)                         # [512, 512]
    pb = proj_b.reshape(4, 128).T                               # [128, 4]

    # gfw column block j (used as lhsT [128, 32] for channel chunk j): maps
    # channel 128j+p to its global group 8j + p//16.
    gfw = np.zeros((128, 128), np.float64)
    for j in range(4):
        for p_ in range(128):
            gfw[p_, 32 * j + 8 * j + p_ // GSZ] = 1.0
    ch = np.arange(C)
    gbw = (ch[None, :] // GSZ == np.arange(G)[:, None]).astype(np.float64)

    import ml_dtypes
    f = np.float32
    bf16 = ml_dtypes.bfloat16
    return dict(ones8=np.ones((128, 8), bf16),
                wqk=np.ascontiguousarray(wqk.astype(bf16)),
                bqk=np.ascontiguousarray(bqk, f),
                wv=np.ascontiguousarray(wv.astype(bf16)),
                bvb=np.ascontiguousarray(bv[None, :], f),
                wp=np.ascontiguousarray(wp.astype(bf16)),
                pb=np.ascontiguousarray(pb, f),
                gfw=np.ascontiguousarray(gfw, f),
                gbw=np.ascontiguousarray(gbw, f))


def kernel(x, norm_w, norm_b, qkv_w, qkv_b, proj_w, proj_b, _trace=False):
    x = np.asarray(x, np.float32)
    consts = _prep_constants(norm_w, norm_b, qkv_w, qkv_b, proj_w, proj_b)
    nc = _build_module()
    in_maps = []
    import ml_dtypes as _md
    for i in range(N_CORES):
        xi = np.ascontiguousarray(x[i].reshape(C, L))
        m = {"xb": np.ascontiguousarray(xi.astype(_md.bfloat16))}
        m.update(consts)
        in_maps.append(m)
    res = run_bass_kernel_spmd(nc, in_maps, core_ids=list(range(N_CORES)),
                               trace=_trace)
    out = np.stack([res.results[i]["out"] for i in range(N_CORES)])
    if _trace:
        _CACHE["last_results"] = res
    return out.reshape(B, C, HH, WW).astype(np.float32)


# revision 56
# speedup vs baseline: 1.0891x; 1.0891x over previous
"""AttentionBlock (GroupNorm + MHA + proj + residual) on 8 Trainium2 cores.

Sharding: data-parallel over batch (b=8, one sample per NeuronCore).
Per-core kernel computes the full block for one sample entirely on-chip:

  x [512, 1024] (bf16) -> GroupNorm(32 groups) -> qkv (bf16 matmuls)
    -> per-head QK^T (K=64, two heads row-tiled into concurrent PE halves)
    -> exp on ScalarE (softmax denominator via an extra ones column in the
       AV matmul's stationary operand)
    -> AV (K=128) -> normalize -> proj + bias + residual -> out [512, 1024]

v2 changes vs the first working kernel:
  - input is bf16-only (residual from the bf16 copy); input DMA halved and
    spread over 4 engine queues so chunks land in parallel
  - GroupNorm rstd via a DVE bit-trick + 2 Newton steps: no Ln/Exp round
    trip on ScalarE, so the whole kernel uses ONE activation table set
    (exp_and_others: copy/identity/exp) -> one 1.3us table load, not three
  - full-chunk stats passes (Sx on ACT with accum, Sxx on DVE with accum)
  - kc-major emission for pair-0 qkv so it pipelines behind the per-chunk
    affine; q/k bias adds split between ACT and DVE
  - softmax normalize: reciprocal straight from the PSUM denominator row,
    gpsimd broadcast, one DVE multiply (numerator read from PSUM) -- no
    ScalarE involvement in the steady state (ACT runs exp back-to-back)
  - pair-3 runs two of its own AV sweeps inside its S loop; the remaining
    two + proj make a short tail
"""
import sys

sys.path.insert(0, "/opt/trn_rl_repo")

import numpy as np

import concourse.bacc as bacc
import concourse.mybir as mybir
from concourse.bass_utils import run_bass_kernel_spmd
from concourse.tile import TileContext

AF = mybir.ActivationFunctionType
OP = mybir.AluOpType
F32 = mybir.dt.float32
U32 = mybir.dt.uint32
BF16 = mybir.dt.bfloat16

B, C, HH, WW = 8, 512, 32, 32
L = HH * WW          # 1024
H = 8                # heads
HD = C // H          # 64
G = 32               # groups
GSZ = C // G         # 16 channels per group
EPS = 1e-5
N_CORES = 8
EXP_BUFS = 32
MAGIC = 0x5F3759DF

_CACHE = {}


def _build_module():
    if "nc" in _CACHE:
        return _CACHE["nc"]
    nc = bacc.Bacc("TRN2", target_bir_lowering=False, debug=False)

    xb_d = nc.dram_tensor("xb", [C, L], BF16, kind="ExternalInput")
    wqk_d = nc.dram_tensor("wqk", [C, 2 * C], BF16, kind="ExternalInput")
    bqk_d = nc.dram_tensor("bqk", [128, 8], F32, kind="ExternalInput")
    wv_d = nc.dram_tensor("wv", [C, C], BF16, kind="ExternalInput")
    bvb_d = nc.dram_tensor("bvb", [1, C], F32, kind="ExternalInput")
    wp_d = nc.dram_tensor("wp", [C, C], BF16, kind="ExternalInput")
    pb_d = nc.dram_tensor("pb", [128, 4], F32, kind="ExternalInput")
    gfw_d = nc.dram_tensor("gfw", [128, 128], F32, kind="ExternalInput")
    gbw_d = nc.dram_tensor("gbw", [G, C], F32, kind="ExternalInput")
    ones8_d = nc.dram_tensor("ones8", [128, 8], BF16, kind="ExternalInput")
    out_d = nc.dram_tensor("out", [C, L], F32, kind="ExternalOutput")
    dbg_d = nc.dram_tensor("dbg", [C, L], F32, kind="ExternalOutput") if _CACHE.get("debug") else None

    with TileContext(nc) as tc:
        with tc.tile_pool(name="persist", bufs=1) as per, \
             tc.tile_pool(name="expp", bufs=EXP_BUFS) as expp, \
             tc.tile_pool(name="outp", bufs=3) as outp, \
             tc.tile_pool(name="small", bufs=4) as smallp, \
             tc.tile_pool(name="sps", bufs=3, space="PSUM") as spp, \
             tc.tile_pool(name="ap4", bufs=2, space="PSUM") as ap4:

            # ---------- persistent tiles ----------
            xbt = [per.tile([128, L], BF16, tag=f"xb{j}", name=f"xb{j}") for j in range(4)]
            xn = [per.tile([128, L], BF16, tag=f"xn{j}", name=f"xn{j}") for j in range(4)]
            a_t = [per.tile([128, L], BF16, tag=f"a{j}", name=f"a{j}") for j in range(4)]
            qp = [per.tile([128, L], BF16, tag=f"qp{j}", name=f"qp{j}") for j in range(4)]
            kp = [per.tile([128, L], BF16, tag=f"kp{j}", name=f"kp{j}") for j in range(4)]
            vt = [per.tile([128, H * (HD + 1)], BF16, tag=f"vt{j}", name=f"vt{j}") for j in range(8)]
            wqk = [per.tile([128, 2 * C], BF16, tag=f"wqk{k}", name=f"wqk{k}") for k in range(4)]
            wv = [per.tile([128, C], BF16, tag=f"wv{k}", name=f"wv{k}") for k in range(4)]
            wp = [per.tile([128, C], BF16, tag=f"wp{k}", name=f"wp{k}") for k in range(4)]
            gfw_t = per.tile([128, 128], F32, tag="gfw", name="gfw")
            gbw_t = per.tile([G, C], F32, tag="gbw", name="gbw")
            bqk_t = per.tile([128, 8], F32, tag="bqk", name="bqk")
            pb_t = per.tile([128, 4], F32, tag="pb", name="pb")
            ones8_t = per.tile([128, 8], BF16, tag="ones8", name="ones8")
            bvr_t = per.tile([1, C], F32, tag="bvr", name="bvr")
            bvb_t = per.tile([128, C], F32, tag="bvb", name="bvb")

            # ---------- input DMAs, spread across engine queues ----------
            # (only SP/Activation/GpSimd can issue DMAs); xb chunks go in
            # halves across queues so stats can start ~2us earlier
            half_eng = {(0, 0): nc.sync, (0, 1): nc.scalar,
                        (1, 0): nc.sync, (1, 1): nc.gpsimd,
                        (2, 0): nc.sync, (2, 1): nc.gpsimd,
                        (3, 0): nc.sync, (3, 1): nc.gpsimd}
            for j in range(2):
                for hf in range(2):
                    half_eng[(j, hf)].dma_start(
                        out=xbt[j][:, 512 * hf:512 * hf + 512],
                        in_=xb_d[128 * j:128 * j + 128, 512 * hf:512 * hf + 512])
            nc.sync.dma_start(out=gfw_t[:, :], in_=gfw_d[:, :])
            for j in range(2, 4):
                for hf in range(2):
                    half_eng[(j, hf)].dma_start(
                        out=xbt[j][:, 512 * hf:512 * hf + 512],
                        in_=xb_d[128 * j:128 * j + 128, 512 * hf:512 * hf + 512])
            nc.sync.dma_start(out=gbw_t[:, :], in_=gbw_d[:, :])
            nc.gpsimd.dma_start(out=bqk_t[:, :], in_=bqk_d[:, :])
            nc.gpsimd.dma_start(out=ones8_t[:, :], in_=ones8_d[:, :])
            nc.gpsimd.dma_start(out=bvr_t[:, :], in_=bvb_d[:, :])
            for k in range(4):
                nc.sync.dma_start(out=wqk[k][:, :],
                                  in_=wqk_d[128 * k:128 * k + 128, :])
            for k in range(4):
                nc.gpsimd.dma_start(out=wv[k][:, :],
                                    in_=wv_d[128 * k:128 * k + 128, :])
            for k in range(4):
                nc.sync.dma_start(out=wp[k][:, :], in_=wp_d[128 * k:128 * k + 128, :])
            nc.sync.dma_start(out=pb_t[:, :], in_=pb_d[:, :])
            nc.gpsimd.partition_broadcast(bvb_t[:, :], bvr_t[:, :], channels=128)
            ones1 = per.tile([1, 128], F32, tag="ones1", name="ones1")
            nc.vector.memset(ones1[:, :], 1.0)

            def warm(n):
                wup = ap4.tile([128, 128], F32, tag="acc", name="acc")
                for _ in range(n):
                    nc.tensor.matmul(wup[:, :], gfw_t[:, :], gfw_t[:, :],
                                     start=True, stop=True)

            warm(10)

            # ---------- GroupNorm stats (per chunk) ----------
            # stats[j][:, 0] = sum_l x, stats[j][:, 1] = sum_l x^2
            stats = [per.tile([128, 2], F32, tag=f"st{j}", name=f"st{j}") for j in range(4)]
            gss = per.tile([G, 2], F32, tag="gss", name="gss")
            gstp = ap4.tile([G, 2], F32, tag="acc", name="acc")
            for j in range(4):
                nc.scalar.activation(out=a_t[j][:, :], in_=xbt[j][:, :],
                                     func=AF.Copy,
                                     accum_out=stats[j][:, 0:1])
                nc.vector.scalar_tensor_tensor(out=xn[j][:, :],
                                               in0=xbt[j][:, :],
                                               scalar=1.0, in1=xbt[j][:, :],
                                               op0=OP.mult, op1=OP.mult,
                                               accum_out=stats[j][:, 1:2])
                nc.tensor.matmul(gstp[:, :], gfw_t[:, 32 * j:32 * j + 32],
                                 stats[j][:, :], start=(j == 0), stop=(j == 3))
            nc.vector.tensor_copy(gss[:, :], gstp[:, :])
            # keep HAM hot through the group-chain + affine windows with
            # matmuls that DEPEND on gss: the scheduler cannot hoist them
            # ahead of the stats->gst chain, so no coalesced wait ever
            # includes them ahead of real work
            wup2 = ap4.tile([128, 2], F32, tag="acc", name="acc")
            for _ in range(12):
                nc.tensor.matmul(wup2[:, :], gbw_t[:, 0:128], gss[:, :],
                                 start=True, stop=True)

            # ---------- group chain: mean/var -> rstd via bit-trick ----------
            mean = per.tile([G, 1], F32, tag="mean", name="mean")
            nmean = per.tile([G, 1], F32, tag="nmean", name="nmean")
            e2e = per.tile([G, 1], F32, tag="e2e", name="e2e")
            veps = per.tile([G, 1], F32, tag="veps", name="veps")
            vh = per.tile([G, 1], F32, tag="vh", name="vh")
            magic = per.tile([G, 1], U32, tag="magic", name="magic")
            c15 = per.tile([G, 1], F32, tag="c15", name="c15")
            ush = per.tile([G, 1], U32, tag="ush", name="ush")
            y = per.tile([G, 1], F32, tag="y0", name="y0")
            t1 = per.tile([G, 1], F32, tag="t1", name="t1")
            u1 = per.tile([G, 1], F32, tag="u1", name="u1")
            y1 = per.tile([G, 1], F32, tag="y1", name="y1")
            t2 = per.tile([G, 1], F32, tag="t2", name="t2")
            u2 = per.tile([G, 1], F32, tag="u2", name="u2")
            gsb = per.tile([G, 2], F32, tag="gsb", name="gsb")

            nc.vector.memset(magic[:, :], MAGIC)
            nc.vector.memset(c15[:, :], 1.5)
            inv_n = 1.0 / (GSZ * L)
            nc.vector.tensor_scalar(out=mean[:, :], in0=gss[:, 0:1],
                                    scalar1=inv_n, scalar2=None, op0=OP.mult)
            nc.vector.tensor_scalar(out=e2e[:, :], in0=gss[:, 1:2],
                                    scalar1=inv_n, scalar2=EPS,
                                    op0=OP.mult, op1=OP.add)
            nc.vector.tensor_scalar(out=nmean[:, :], in0=mean[:, :],
                                    scalar1=-1.0, scalar2=None, op0=OP.mult)
            # veps = e2e - mean^2 = (mean * nmean) + e2e
            nc.vector.scalar_tensor_tensor(out=veps[:, :], in0=mean[:, :],
                                           scalar=nmean[:, 0:1], in1=e2e[:, :],
                                           op0=OP.mult, op1=OP.add)
            # rsqrt seed: y = bitcast(MAGIC - (bitcast(veps) >> 1))
            nc.vector.tensor_scalar(out=ush[:, :], in0=veps[:, :].bitcast(U32),
                                    scalar1=1, scalar2=None,
                                    op0=OP.logical_shift_right)
            nc.vector.scalar_tensor_tensor(out=y[:, :].bitcast(U32),
                                           in0=magic[:, :], scalar=0,
                                           in1=ush[:, :],
                                           op0=OP.bypass, op1=OP.subtract)
            nc.vector.tensor_scalar(out=vh[:, :], in0=veps[:, :],
                                    scalar1=0.5, scalar2=None, op0=OP.mult)
            # Newton 1: y1 = -(y * (1.5 - 0.5 v y^2))
            nc.vector.tensor_tensor(out=t1[:, :], in0=y[:, :], in1=y[:, :],
                                    op=OP.mult)
            nc.vector.scalar_tensor_tensor(out=u1[:, :], in0=t1[:, :],
                                           scalar=vh[:, 0:1], in1=c15[:, :],
                                           op0=OP.mult, op1=OP.subtract)
            nc.vector.tensor_tensor(out=y1[:, :], in0=y[:, :], in1=u1[:, :],
                                    op=OP.mult)
            # Newton 2: y2 = y1m * -(1.5 - 0.5 v y1^2)  (signs cancel)
            nc.vector.tensor_tensor(out=t2[:, :], in0=y1[:, :], in1=y1[:, :],
                                    op=OP.mult)
            nc.vector.scalar_tensor_tensor(out=u2[:, :], in0=t2[:, :],
                                           scalar=vh[:, 0:1], in1=c15[:, :],
                                           op0=OP.mult, op1=OP.subtract)
            nc.vector.tensor_tensor(out=gsb[:, 0:1], in0=y1[:, :], in1=u2[:, :],
                                    op=OP.mult)
            nc.vector.tensor_tensor(out=gsb[:, 1:2], in0=nmean[:, :],
                                    in1=gsb[:, 0:1], op=OP.mult)

            # ---------- per-channel affine coefficients + apply ----------
            cb = [per.tile([128, 2], F32, tag=f"cb{j}", name=f"cb{j}") for j in range(4)]
            for j in range(4):
                cbp = ap4.tile([128, 2], F32, tag="acc", name="acc")
                nc.tensor.matmul(cbp[:, :], gbw_t[:, 128 * j:128 * j + 128],
                                 gsb[:, :], start=True, stop=True)
                nc.vector.tensor_copy(cb[j][:, :], cbp[:, :])
                if j % 2 == 0:
                    nc.scalar.activation(out=xn[j][:, :], in_=xbt[j][:, :],
                                         func=AF.Identity,
                                         bias=cb[j][:, 1:2], scale=cb[j][:, 0:1])
                else:
                    nc.vector.tensor_scalar(out=xn[j][:, :], in0=xbt[j][:, :],
                                            scalar1=cb[j][:, 0:1],
                                            scalar2=cb[j][:, 1:2],
                                            op0=OP.mult, op1=OP.add)

            # ---------- qkv pair 0: kc-major so it pipelines behind affine ----
            # (4 concurrently-open accumulators: 2 from ap4, 2 from the
            # not-yet-used S pool)
            pq0 = {}
            for kc in range(4):
                for gi, (m, n2) in enumerate([(0, 0), (0, 1), (4, 0), (4, 1)]):
                    if kc == 0:
                        pool = ap4 if gi < 2 else spp
                        pq0[gi] = pool.tile([128, 512], F32, tag="acc" if gi < 2 else "sps",
                                            name="acc" if gi < 2 else "sps")
                    nc.tensor.matmul(pq0[gi][:, :],
                                     wqk[kc][:, 128 * m:128 * m + 128],
                                     xn[kc][:, 512 * n2:512 * n2 + 512],
                                     start=(kc == 0), stop=(kc == 3))
            # bias adds: split ACT/DVE; n2=0 halves first so S(sc=0) can start
            for gi, (m, n2) in enumerate([(4, 0), (0, 0), (0, 1), (4, 1)]):
                dest = qp[0] if m < 4 else kp[0]
                dsl = dest[:, 512 * n2:512 * n2 + 512]
                if gi % 2 == 0:
                    nc.scalar.activation(out=dsl, in_=pq0[[2, 0, 1, 3][gi]][:, :],
                                         func=AF.Identity,
                                         bias=bqk_t[:, m:m + 1])
                else:
                    nc.vector.tensor_scalar(out=dsl, in0=pq0[[2, 0, 1, 3][gi]][:, :],
                                            scalar1=bqk_t[:, m:m + 1],
                                            scalar2=None, op0=OP.add)

            # ---------- helpers ----------
            class QkvStream:
                """qkv chunks for pairs 1-3 as an emit-on-demand stream.
                n2=0 halves for both q and k come first: the next pair's
                first S chunks only need them (k's n2=1 half is first read
                at s-chunk 4)."""
                def __init__(self, ms):
                    self.jobs = [(m, n2) for n2 in range(2) for m in ms]
                    self.i = 0
                    self.pq = None

                def emit(self, k):
                    for _ in range(k):
                        if self.i >= 4 * len(self.jobs):
                            return
                        job, kc = divmod(self.i, 4)
                        m, n2 = self.jobs[job]
                        if kc == 0:
                            self.pq = ap4.tile([128, 512], F32, tag="acc",
                                               name="acc")
                        nc.tensor.matmul(self.pq[:, :],
                                         wqk[kc][:, 128 * m:128 * m + 128],
                                         xn[kc][:, 512 * n2:512 * n2 + 512],
                                         start=(kc == 0), stop=(kc == 3))
                        if kc == 3:
                            dest = qp[m % 4] if m < 4 else kp[m - 4]
                            nc.vector.tensor_scalar(
                                out=dest[:, 512 * n2:512 * n2 + 512],
                                in0=self.pq[:, :],
                                scalar1=bqk_t[:, m:m + 1], scalar2=None,
                                op0=OP.add)
                        self.i += 1

            def vt_chunk(sc):
                """v^T for s-chunk sc, all heads: [128 s, 8*(64+1)] with a
                ones column per head (softmax denominator accumulator)."""
                pv = ap4.tile([128, 512], F32, tag="acc", name="acc")
                for kc in range(4):
                    nc.tensor.matmul(pv[:, :],
                                     xn[kc][:, 128 * sc:128 * sc + 128],
                                     wv[kc][:, :], start=(kc == 0), stop=(kc == 3))
                v3 = vt[sc][:, :].rearrange("p (h e) -> p h e", e=HD + 1)
                nc.vector.tensor_copy(vt[sc][:, HD::HD + 1], ones8_t[:, :])
                nc.vector.tensor_tensor(
                    out=v3[:, :, 0:HD],
                    in0=pv[:, :].rearrange("p (h e) -> p h e", e=HD),
                    in1=bvb_t[:, :].rearrange("p (h e) -> p h e", e=HD),
                    op=OP.add)

            def norm_head(p, e, n2, pa, act_copy=False):
                """softmax-normalize one AV accumulator into a_t.  The PSUM
                accumulator is drained immediately (denominator row + raw
                numerator) so its slot recycles fast; the normalization then
                runs SBUF-side in place.  act_copy routes the drain copies
                to ScalarE (for tail norms, when it has no exp work left)."""
                base = 64 * e
                asl = a_t[p][base:base + 64, 512 * n2:512 * n2 + 512]
                dsb = smallp.tile([1, 512], F32, tag="dsb", name="dsb")
                if act_copy:
                    nc.scalar.copy(dsb[:, :], pa[HD:HD + 1, :])
                    nc.scalar.copy(asl, pa[0:HD, :])
                else:
                    nc.vector.tensor_copy(dsb[:, :], pa[HD:HD + 1, :])
                    nc.vector.tensor_copy(asl, pa[0:HD, :])
                rcp = smallp.tile([1, 512], F32, tag="rcp", name="rcp")
                nc.vector.reciprocal_approx_fast(out=rcp[:, :],
                                                 in_=dsb[:, :])
                db = smallp.tile([128, 512], F32, tag="db", name="db")
                nc.gpsimd.partition_broadcast(db[:, :], rcp[:, :],
                                              channels=128)
                nc.vector.tensor_tensor(out=asl, in0=asl,
                                        in1=db[base:base + 64, :],
                                        op=OP.mult)

            class AvStream:
                """AV accumulation sweeps as an emit-on-demand stream
                (8 matmuls per sweep; norm emitted when a sweep closes).
                lockstep=True advances all sweeps one s-chunk at a time so a
                pair's own sweeps can ride its S loop, gated only on the
                exp tiles already produced."""
                def __init__(self, pe, sweeps, lockstep=False, act_copy=False,
                             pool=None):
                    self.p, self.est = pe
                    self.sweeps = sweeps
                    self.lockstep = lockstep
                    self.act_copy = act_copy
                    self.pool = pool or ap4
                    self.i = 0
                    self.pa = [None] * len(self.sweeps)

                def emit(self, k):
                    for _ in range(k):
                        if self.i >= 8 * len(self.sweeps):
                            return
                        if self.lockstep:
                            sweep = self.i % len(self.sweeps)
                            sc = self.i // len(self.sweeps)
                        else:
                            sweep, sc = divmod(self.i, 8)
                        e, n2 = self.sweeps[sweep]
                        h = 2 * self.p + e
                        if sc == 0:
                            self.pa[sweep] = self.pool.tile(
                                [HD + 1, 512], F32,
                                tag="acc" if self.pool is ap4 else "sps",
                                name="acc" if self.pool is ap4 else "sps")
                        nc.tensor.matmul(
                            self.pa[sweep][:, :], vt[sc][:, 65 * h:65 * h + 65],
                            self.est[e][sc][:, 512 * n2:512 * n2 + 512],
                            start=(sc == 0), stop=(sc == 7))
                        if sc == 7:
                            norm_head(self.p, e, n2, self.pa[sweep],
                                      act_copy=self.act_copy)
                        self.i += 1

            def s_mm(p, e, sc, est):
                """one head's S^T chunk + its exp"""
                base = 64 * e
                ps_s = spp.tile([128, L], F32, tag="sps", name="sps")
                for n2 in range(2):
                    nc.tensor.matmul(
                        ps_s[:, 512 * n2:512 * n2 + 512],
                        kp[p][base:base + 64, 128 * sc:128 * sc + 128],
                        qp[p][base:base + 64, 512 * n2:512 * n2 + 512],
                        start=True, stop=True, tile_position=(base, 0))
                es = expp.tile([128, L], BF16, tag="expS", name="expS")
                nc.scalar.activation(out=es[:, :], in_=ps_s[:, :], func=AF.Exp)
                est[e][sc] = es

            def attn_S(p, prev=None, qkv=None, own=None, stream_vt=False):
                """S^T + exp for pair p; the previous pair's AV sweeps, pair
                p+1's qkv, and (for p=3) the pair's own first sweep ride
                along ahead of the S matmuls.  The e=0/e=1 S matmul pairs
                stay adjacent so their disjoint row-groups execute
                concurrently on the PE; with 3 S-PSUM slots they are gated
                by the exp three allocations back, which has always
                drained."""
                est = [[None] * 8, [None] * 8]
                if own is not None:
                    own.est = est
                for sc in range(8):
                    s_mm(p, 0, sc, est)
                    s_mm(p, 1, sc, est)
                    if own is not None and sc >= 1:
                        own.emit(1)
                    if prev is not None:
                        prev.emit(4)
                    if stream_vt:
                        vt_chunk(sc)
                    if qkv is not None:
                        qkv.emit(2)
                return est

            # ---------- emission schedule ----------
            prev = None
            own3 = None
            for p in range(4):
                qs = QkvStream([p + 1, p + 5]) if p + 1 < 4 else None
                own3 = AvStream((p, None), sweeps=[(0, 0)],
                                lockstep=True) if p == 3 else None
                est_cur = attn_S(p, prev, qs, own=own3, stream_vt=(p == 0))
                if prev is not None:
                    prev.emit(32)  # drain previous pair's sweeps
                if qs is not None:
                    qs.emit(16)    # drain qkv remainder
                prev = AvStream((p, est_cur),
                                sweeps=[(0, 0), (1, 0), (0, 1), (1, 1)])
            own3.emit(16)          # drain pair 3's first sweep + its norm
            # tail sweeps use the now-free S PSUM slots so all three can be
            # in flight at once instead of serializing through 2 slots
            av_rest = AvStream((3, prev.est),
                               sweeps=[(1, 0), (1, 1), (0, 1)], act_copy=True,
                               pool=spp)

            class ProjStream:
                """proj groups (m, n2): 4 accumulating matmuls then fused
                bias+residual and the output DMA.  body(m) emits the three
                cc<3 matmuls (gated only on pairs 0-2, long since ready);
                close(m) emits the cc=3 matmul (gated on pair 3's norms) and
                the drain.  The drain alternates DVE and ScalarE."""
                def __init__(self, n2):
                    self.n2 = n2
                    self.po = {}

                def body(self, m):
                    self.po[m] = spp.tile([128, 512], F32, tag="sps",
                                          name="sps")
                    for cc in range(3):
                        nc.tensor.matmul(self.po[m][:, :],
                                         wp[cc][:, 128 * m:128 * m + 128],
                                         a_t[cc][:, 512 * self.n2:512 * self.n2 + 512],
                                         start=(cc == 0), stop=False)

                def close(self, m, act=False):
                    n2 = self.n2
                    nc.tensor.matmul(self.po[m][:, :],
                                     wp[3][:, 128 * m:128 * m + 128],
                                     a_t[3][:, 512 * n2:512 * n2 + 512],
                                     start=False, stop=True)
                    ob = outp.tile([128, 512], F32, tag="ob", name="ob")
                    osl = out_d[128 * m:128 * m + 128, 512 * n2:512 * n2 + 512]
                    # ScalarE drains psum+bias (fast slot release, off the
                    # congested DVE); DVE then adds the residual in place
                    nc.scalar.activation(out=ob[:, :], in_=self.po[m][:, :],
                                         func=AF.Identity,
                                         bias=pb_t[:, m:m + 1])
                    nc.vector.tensor_tensor(
                        out=ob[:, :], in0=ob[:, :],
                        in1=xbt[m][:, 512 * n2:512 * n2 + 512], op=OP.add)
                    nc.sync.dma_start(out=osl, in_=ob[:, :])

            if dbg_d is not None:
                dbg_src = _CACHE.get("debug_src", "xn")
                src = {"xn": xn, "qp": qp, "kp": kp, "at": a_t}[dbg_src]
                for j in range(4):
                    dcp = outp.tile([128, L], F32, tag="dbg", name="dbg")
                    nc.vector.tensor_copy(dcp[:, :], src[j][:, :])
                    nc.sync.dma_start(out=dbg_d[128 * j:128 * j + 128, :],
                                      in_=dcp[:, :])

            # tail: pair 3's remaining sweeps interleaved with proj bodies
            # (only pairs 0-2 needed) so the PE never idles; group closes
            # follow as pair 3's norms land.  Allocation order matches the
            # release order of the 3 S-pool slots to avoid FIFO deadlock.
            pr0, pr1 = ProjStream(0), ProjStream(1)
            av_rest.emit(8)    # (1,0)
            av_rest.emit(8)    # (1,1)
            pr0.body(0)
            av_rest.emit(8)    # (0,1)
            pr0.body(1)
            pr0.close(0)
            pr0.body(2)
            pr0.close(1)
            pr0.body(3)
            pr0.close(2)
            pr0.close(3)
            pr1.body(0)
            pr1.body(1)
            pr1.body(2)
            pr1.close(0)
            pr1.body(3)
            pr1.close(1)
            pr1.close(2)
            pr1.close(3)

    nc.compile()
    _CACHE["nc"] = nc
    return nc


def _prep_constants(norm_w, norm_b, qkv_w, qkv_b, proj_w, proj_b):
    norm_w = np.asarray(norm_w, np.float64)
    norm_b = np.asarray(norm_b, np.float64)
    qkv_w = np.asarray(qkv_w, np.float64)
    qkv_b = np.asarray(qkv_b, np.float64)
    proj_w = np.asarray(proj_w, np.float64)
    proj_b = np.asarray(proj_b, np.float64)

    idx = np.arange(HD)
    q_idx = np.concatenate([h * 3 * HD + idx for h in range(H)])
    k_idx = q_idx + HD
    v_idx = q_idx + 2 * HD

    # fold norm affine: qkv = W @ (gn*nw + nb) + b = (W*nw) @ gn + (W@nb + b)
    Wf = qkv_w * norm_w[None, :]
    bf = qkv_b + qkv_w @ norm_b
    s2 = 1.0 / np.sqrt(HD)  # both q*scale and k*scale -> fold s^2 into q
    Wq, bq = Wf[q_idx] * s2, bf[q_idx] * s2
    Wk, bk = Wf[k_idx], bf[k_idx]
    Wv, bv = Wf[v_idx], bf[v_idx]

    wqk = np.concatenate([Wq.T, Wk.T], axis=1)                  # [512, 1024]
    bqk = np.concatenate([bq, bk]).reshape(8, 128).T            # [128, 8]
    wv = np.ascontiguousarray(Wv.T)                             # [512, 512]
    wp = np.ascontiguousarray(proj_w.T)                         # [512, 512]
    pb = proj_b.reshape(4, 128).T                               # [128, 4]

    # gfw column block j (used as lhsT [128, 32] for channel chunk j): maps
    # channel 128j+p to its global group 8j + p//16.
    gfw = np.zeros((128, 128), np.float64)
    for j in range(4):
        for p_ in range(128):
            gfw[p_, 32 * j + 8 * j + p_ // GSZ] = 1.0
    ch = np.arange(C)
    gbw = (ch[None, :] // GSZ == np.arange(G)[:, None]).astype(np.float64)

    import ml_dtypes
    f = np.float32
    bf16 = ml_dtypes.bfloat16
    return dict(ones8=np.ones((128, 8), bf16),
                wqk=np.ascontiguousarray(wqk.astype(bf16)),
                bqk=np.ascontiguousarray(bqk, f),
                wv=np.ascontiguousarray(wv.astype(bf16)),
                bvb=np.ascontiguousarray(bv[None, :], f),
                wp=np.ascontiguousarray(wp.astype(bf16)),
                pb=np.ascontiguousarray(pb, f),
                gfw=np.ascontiguousarray(gfw, f),
                gbw=np.ascontiguousarray(gbw, f))


def kernel(x, norm_w, norm_b, qkv_w, qkv_b, proj_w, proj_b, _trace=False):
    x = np.asarray(x, np.float32)
    consts = _prep_constants(norm_w, norm_b, qkv_w, qkv_b, proj_w, proj_b)
    nc = _build_module()
    in_maps = []
    import ml_dtypes as _md
    for i in range(N_CORES):
        xi = np.ascontiguousarray(x[i].reshape(C, L))
        m = {"xb": np.ascontiguousarray(xi.astype(_md.bfloat16))}
        m.update(consts)
        in_maps.append(m)
    res = run_bass_kernel_spmd(nc, in_maps, core_ids=list(range(N_CORES)),
                               trace=_trace)
    out = np.stack([res.results[i]["out"] for i in range(N_CORES)])
    if _trace:
        _CACHE["last_results"] = res
    return out.reshape(B, C, HH, WW).astype(np.float32)


# revision 58
# speedup vs baseline: 1.1226x; 1.0307x over previous
"""AttentionBlock (GroupNorm + MHA + proj + residual) on 8 Trainium2 cores.

Sharding: data-parallel over batch (b=8, one sample per NeuronCore).
Per-core kernel computes the full block for one sample entirely on-chip:

  x [512, 1024] (bf16) -> GroupNorm(32 groups) -> qkv (bf16 matmuls)
    -> per-head QK^T (K=64, two heads row-tiled into concurrent PE halves)
    -> exp on ScalarE (softmax denominator via an extra ones column in the
       AV matmul's stationary operand)
    -> AV (K=128) -> normalize -> proj + bias + residual -> out [512, 1024]

v2 changes vs the first working kernel:
  - input is bf16-only (residual from the bf16 copy); input DMA halved and
    spread over 4 engine queues so chunks land in parallel
  - GroupNorm rstd via a DVE bit-trick + 2 Newton steps: no Ln/Exp round
    trip on ScalarE, so the whole kernel uses ONE activation table set
    (exp_and_others: copy/identity/exp) -> one 1.3us table load, not three
  - full-chunk stats passes (Sx on ACT with accum, Sxx on DVE with accum)
  - kc-major emission for pair-0 qkv so it pipelines behind the per-chunk
    affine; q/k bias adds split between ACT and DVE
  - softmax normalize: reciprocal straight from the PSUM denominator row,
    gpsimd broadcast, one DVE multiply (numerator read from PSUM) -- no
    ScalarE involvement in the steady state (ACT runs exp back-to-back)
  - pair-3 runs two of its own AV sweeps inside its S loop; the remaining
    two + proj make a short tail
"""
import sys

sys.path.insert(0, "/opt/trn_rl_repo")

import numpy as np

import concourse.bacc as bacc
import concourse.mybir as mybir
from concourse.bass_utils import run_bass_kernel_spmd
from concourse.tile import TileContext

AF = mybir.ActivationFunctionType
OP = mybir.AluOpType
F32 = mybir.dt.float32
U32 = mybir.dt.uint32
BF16 = mybir.dt.bfloat16

B, C, HH, WW = 8, 512, 32, 32
L = HH * WW          # 1024
H = 8                # heads
HD = C // H          # 64
G = 32               # groups
GSZ = C // G         # 16 channels per group
EPS = 1e-5
N_CORES = 8
EXP_BUFS = 32
MAGIC = 0x5F3759DF

_CACHE = {}


def _build_module():
    if "nc" in _CACHE:
        return _CACHE["nc"]
    nc = bacc.Bacc("TRN2", target_bir_lowering=False, debug=False)

    xb_d = nc.dram_tensor("xb", [C, L], BF16, kind="ExternalInput")
    wqk_d = nc.dram_tensor("wqk", [C, 2 * C], BF16, kind="ExternalInput")
    bqk_d = nc.dram_tensor("bqk", [128, 8], F32, kind="ExternalInput")
    wv_d = nc.dram_tensor("wv", [C, C], BF16, kind="ExternalInput")
    bvb_d = nc.dram_tensor("bvb", [1, C], F32, kind="ExternalInput")
    wp_d = nc.dram_tensor("wp", [C, C], BF16, kind="ExternalInput")
    pb_d = nc.dram_tensor("pb", [128, 4], F32, kind="ExternalInput")
    gfw_d = nc.dram_tensor("gfw", [128, 128], F32, kind="ExternalInput")
    gbw_d = nc.dram_tensor("gbw", [G, C], F32, kind="ExternalInput")
    ones8_d = nc.dram_tensor("ones8", [128, 8], BF16, kind="ExternalInput")
    out_d = nc.dram_tensor("out", [C, L], F32, kind="ExternalOutput")
    dbg_d = nc.dram_tensor("dbg", [C, L], F32, kind="ExternalOutput") if _CACHE.get("debug") else None

    with TileContext(nc) as tc:
        with tc.tile_pool(name="persist", bufs=1) as per, \
             tc.tile_pool(name="expp", bufs=EXP_BUFS) as expp, \
             tc.tile_pool(name="outp", bufs=6) as outp, \
             tc.tile_pool(name="small", bufs=4) as smallp, \
             tc.tile_pool(name="sps", bufs=3, space="PSUM") as spp, \
             tc.tile_pool(name="ap4", bufs=2, space="PSUM") as ap4:

            # ---------- persistent tiles ----------
            xbt = [per.tile([128, L], BF16, tag=f"xb{j}", name=f"xb{j}") for j in range(4)]
            xn = [per.tile([128, L], BF16, tag=f"xn{j}", name=f"xn{j}") for j in range(4)]
            a_t = [per.tile([128, L], BF16, tag=f"a{j}", name=f"a{j}") for j in range(4)]
            qp = [per.tile([128, L], BF16, tag=f"qp{j}", name=f"qp{j}") for j in range(4)]
            kp = [per.tile([128, L], BF16, tag=f"kp{j}", name=f"kp{j}") for j in range(4)]
            vt = [per.tile([128, H * (HD + 1)], BF16, tag=f"vt{j}", name=f"vt{j}") for j in range(8)]
            wqk = [per.tile([128, 2 * C], BF16, tag=f"wqk{k}", name=f"wqk{k}") for k in range(4)]
            wv = [per.tile([128, C], BF16, tag=f"wv{k}", name=f"wv{k}") for k in range(4)]
            wp = [per.tile([128, C], BF16, tag=f"wp{k}", name=f"wp{k}") for k in range(4)]
            gfw_t = per.tile([128, 128], F32, tag="gfw", name="gfw")
            gbw_t = per.tile([G, C], F32, tag="gbw", name="gbw")
            bqk_t = per.tile([128, 8], F32, tag="bqk", name="bqk")
            pb_t = per.tile([128, 4], F32, tag="pb", name="pb")
            ones8_t = per.tile([128, 8], BF16, tag="ones8", name="ones8")
            bvr_t = per.tile([1, C], F32, tag="bvr", name="bvr")
            bvb_t = per.tile([128, C], F32, tag="bvb", name="bvb")

            # ---------- input DMAs, spread across engine queues ----------
            # (only SP/Activation/GpSimd can issue DMAs); xb chunks go in
            # halves across queues so stats can start ~2us earlier
            half_eng = {(0, 0): nc.sync, (0, 1): nc.scalar,
                        (1, 0): nc.sync, (1, 1): nc.gpsimd,
                        (2, 0): nc.sync, (2, 1): nc.gpsimd,
                        (3, 0): nc.sync, (3, 1): nc.gpsimd}
            for j in range(2):
                for hf in range(2):
                    half_eng[(j, hf)].dma_start(
                        out=xbt[j][:, 512 * hf:512 * hf + 512],
                        in_=xb_d[128 * j:128 * j + 128, 512 * hf:512 * hf + 512])
            nc.sync.dma_start(out=gfw_t[:, :], in_=gfw_d[:, :])
            for j in range(2, 4):
                for hf in range(2):
                    half_eng[(j, hf)].dma_start(
                        out=xbt[j][:, 512 * hf:512 * hf + 512],
                        in_=xb_d[128 * j:128 * j + 128, 512 * hf:512 * hf + 512])
            nc.sync.dma_start(out=gbw_t[:, :], in_=gbw_d[:, :])
            nc.gpsimd.dma_start(out=bqk_t[:, :], in_=bqk_d[:, :])
            nc.gpsimd.dma_start(out=ones8_t[:, :], in_=ones8_d[:, :])
            nc.gpsimd.dma_start(out=bvr_t[:, :], in_=bvb_d[:, :])
            for k in range(4):
                nc.sync.dma_start(out=wqk[k][:, :],
                                  in_=wqk_d[128 * k:128 * k + 128, :])
            for k in range(4):
                nc.gpsimd.dma_start(out=wv[k][:, :],
                                    in_=wv_d[128 * k:128 * k + 128, :])
            for k in range(4):
                nc.sync.dma_start(out=wp[k][:, :], in_=wp_d[128 * k:128 * k + 128, :])
            nc.sync.dma_start(out=pb_t[:, :], in_=pb_d[:, :])
            nc.gpsimd.partition_broadcast(bvb_t[:, :], bvr_t[:, :], channels=128)
            ones1 = per.tile([1, 128], F32, tag="ones1", name="ones1")
            nc.vector.memset(ones1[:, :], 1.0)

            def warm(n):
                wup = ap4.tile([128, 128], F32, tag="acc", name="acc")
                for _ in range(n):
                    nc.tensor.matmul(wup[:, :], gfw_t[:, :], gfw_t[:, :],
                                     start=True, stop=True)

            warm(10)

            # ---------- GroupNorm stats (per chunk) ----------
            # stats[j][:, 0] = sum_l x, stats[j][:, 1] = sum_l x^2
            stats = [per.tile([128, 2], F32, tag=f"st{j}", name=f"st{j}") for j in range(4)]
            gss = per.tile([G, 2], F32, tag="gss", name="gss")
            gstp = ap4.tile([G, 2], F32, tag="acc", name="acc")
            for j in range(4):
                nc.scalar.activation(out=a_t[j][:, :], in_=xbt[j][:, :],
                                     func=AF.Copy,
                                     accum_out=stats[j][:, 0:1])
                nc.vector.scalar_tensor_tensor(out=xn[j][:, :],
                                               in0=xbt[j][:, :],
                                               scalar=1.0, in1=xbt[j][:, :],
                                               op0=OP.mult, op1=OP.mult,
                                               accum_out=stats[j][:, 1:2])
                nc.tensor.matmul(gstp[:, :], gfw_t[:, 32 * j:32 * j + 32],
                                 stats[j][:, :], start=(j == 0), stop=(j == 3))
            nc.vector.tensor_copy(gss[:, :], gstp[:, :])
            # keep HAM hot through the group-chain + affine windows with
            # matmuls that DEPEND on gss: the scheduler cannot hoist them
            # ahead of the stats->gst chain, so no coalesced wait ever
            # includes them ahead of real work
            wup2 = ap4.tile([128, 2], F32, tag="acc", name="acc")
            for _ in range(12):
                nc.tensor.matmul(wup2[:, :], gbw_t[:, 0:128], gss[:, :],
                                 start=True, stop=True)

            # ---------- group chain: mean/var -> rstd via bit-trick ----------
            mean = per.tile([G, 1], F32, tag="mean", name="mean")
            nmean = per.tile([G, 1], F32, tag="nmean", name="nmean")
            e2e = per.tile([G, 1], F32, tag="e2e", name="e2e")
            veps = per.tile([G, 1], F32, tag="veps", name="veps")
            vh = per.tile([G, 1], F32, tag="vh", name="vh")
            magic = per.tile([G, 1], U32, tag="magic", name="magic")
            c15 = per.tile([G, 1], F32, tag="c15", name="c15")
            ush = per.tile([G, 1], U32, tag="ush", name="ush")
            y = per.tile([G, 1], F32, tag="y0", name="y0")
            t1 = per.tile([G, 1], F32, tag="t1", name="t1")
            u1 = per.tile([G, 1], F32, tag="u1", name="u1")
            y1 = per.tile([G, 1], F32, tag="y1", name="y1")
            t2 = per.tile([G, 1], F32, tag="t2", name="t2")
            u2 = per.tile([G, 1], F32, tag="u2", name="u2")
            gsb = per.tile([G, 2], F32, tag="gsb", name="gsb")

            nc.vector.memset(magic[:, :], MAGIC)
            nc.vector.memset(c15[:, :], 1.5)
            inv_n = 1.0 / (GSZ * L)
            nc.vector.tensor_scalar(out=mean[:, :], in0=gss[:, 0:1],
                                    scalar1=inv_n, scalar2=None, op0=OP.mult)
            nc.vector.tensor_scalar(out=e2e[:, :], in0=gss[:, 1:2],
                                    scalar1=inv_n, scalar2=EPS,
                                    op0=OP.mult, op1=OP.add)
            nc.vector.tensor_scalar(out=nmean[:, :], in0=mean[:, :],
                                    scalar1=-1.0, scalar2=None, op0=OP.mult)
            # veps = e2e - mean^2 = (mean * nmean) + e2e
            nc.vector.scalar_tensor_tensor(out=veps[:, :], in0=mean[:, :],
                                           scalar=nmean[:, 0:1], in1=e2e[:, :],
                                           op0=OP.mult, op1=OP.add)
            # rsqrt seed: y = bitcast(MAGIC - (bitcast(veps) >> 1))
            nc.vector.tensor_scalar(out=ush[:, :], in0=veps[:, :].bitcast(U32),
                                    scalar1=1, scalar2=None,
                                    op0=OP.logical_shift_right)
            nc.vector.scalar_tensor_tensor(out=y[:, :].bitcast(U32),
                                           in0=magic[:, :], scalar=0,
                                           in1=ush[:, :],
                                           op0=OP.bypass, op1=OP.subtract)
            nc.vector.tensor_scalar(out=vh[:, :], in0=veps[:, :],
                                    scalar1=0.5, scalar2=None, op0=OP.mult)
            # Newton 1: y1 = -(y * (1.5 - 0.5 v y^2))
            nc.vector.tensor_tensor(out=t1[:, :], in0=y[:, :], in1=y[:, :],
                                    op=OP.mult)
            nc.vector.scalar_tensor_tensor(out=u1[:, :], in0=t1[:, :],
                                           scalar=vh[:, 0:1], in1=c15[:, :],
                                           op0=OP.mult, op1=OP.subtract)
            nc.vector.tensor_tensor(out=y1[:, :], in0=y[:, :], in1=u1[:, :],
                                    op=OP.mult)
            # Newton 2: y2 = y1m * -(1.5 - 0.5 v y1^2)  (signs cancel)
            nc.vector.tensor_tensor(out=t2[:, :], in0=y1[:, :], in1=y1[:, :],
                                    op=OP.mult)
            nc.vector.scalar_tensor_tensor(out=u2[:, :], in0=t2[:, :],
                                           scalar=vh[:, 0:1], in1=c15[:, :],
                                           op0=OP.mult, op1=OP.subtract)
            nc.vector.tensor_tensor(out=gsb[:, 0:1], in0=y1[:, :], in1=u2[:, :],
                                    op=OP.mult)
            nc.vector.tensor_tensor(out=gsb[:, 1:2], in0=nmean[:, :],
                                    in1=gsb[:, 0:1], op=OP.mult)

            # ---------- per-channel affine coefficients + apply ----------
            cb = [per.tile([128, 2], F32, tag=f"cb{j}", name=f"cb{j}") for j in range(4)]
            for j in range(4):
                cbp = ap4.tile([128, 2], F32, tag="acc", name="acc")
                nc.tensor.matmul(cbp[:, :], gbw_t[:, 128 * j:128 * j + 128],
                                 gsb[:, :], start=True, stop=True)
                nc.vector.tensor_copy(cb[j][:, :], cbp[:, :])
                if j % 2 == 0:
                    nc.scalar.activation(out=xn[j][:, :], in_=xbt[j][:, :],
                                         func=AF.Identity,
                                         bias=cb[j][:, 1:2], scale=cb[j][:, 0:1])
                else:
                    nc.vector.tensor_scalar(out=xn[j][:, :], in0=xbt[j][:, :],
                                            scalar1=cb[j][:, 0:1],
                                            scalar2=cb[j][:, 1:2],
                                            op0=OP.mult, op1=OP.add)

            # ---------- qkv pair 0: kc-major so it pipelines behind affine ----
            # (4 concurrently-open accumulators: 2 from ap4, 2 from the
            # not-yet-used S pool)
            pq0 = {}
            for kc in range(4):
                for gi, (m, n2) in enumerate([(0, 0), (0, 1), (4, 0), (4, 1)]):
                    if kc == 0:
                        pool = ap4 if gi < 2 else spp
                        pq0[gi] = pool.tile([128, 512], F32, tag="acc" if gi < 2 else "sps",
                                            name="acc" if gi < 2 else "sps")
                    nc.tensor.matmul(pq0[gi][:, :],
                                     wqk[kc][:, 128 * m:128 * m + 128],
                                     xn[kc][:, 512 * n2:512 * n2 + 512],
                                     start=(kc == 0), stop=(kc == 3))
            # bias adds: split ACT/DVE; n2=0 halves first so S(sc=0) can start
            for gi, (m, n2) in enumerate([(4, 0), (0, 0), (0, 1), (4, 1)]):
                dest = qp[0] if m < 4 else kp[0]
                dsl = dest[:, 512 * n2:512 * n2 + 512]
                if gi % 2 == 0:
                    nc.scalar.activation(out=dsl, in_=pq0[[2, 0, 1, 3][gi]][:, :],
                                         func=AF.Identity,
                                         bias=bqk_t[:, m:m + 1])
                else:
                    nc.vector.tensor_scalar(out=dsl, in0=pq0[[2, 0, 1, 3][gi]][:, :],
                                            scalar1=bqk_t[:, m:m + 1],
                                            scalar2=None, op0=OP.add)

            # ---------- helpers ----------
            class QkvStream:
                """qkv chunks for pairs 1-3 as an emit-on-demand stream.
                n2=0 halves for both q and k come first: the next pair's
                first S chunks only need them (k's n2=1 half is first read
                at s-chunk 4)."""
                def __init__(self, ms):
                    self.jobs = [(m, n2) for n2 in range(2) for m in ms]
                    self.i = 0
                    self.pq = None

                def emit(self, k):
                    for _ in range(k):
                        if self.i >= 4 * len(self.jobs):
                            return
                        job, kc = divmod(self.i, 4)
                        m, n2 = self.jobs[job]
                        if kc == 0:
                            self.pq = ap4.tile([128, 512], F32, tag="acc",
                                               name="acc")
                        nc.tensor.matmul(self.pq[:, :],
                                         wqk[kc][:, 128 * m:128 * m + 128],
                                         xn[kc][:, 512 * n2:512 * n2 + 512],
                                         start=(kc == 0), stop=(kc == 3))
                        if kc == 3:
                            dest = qp[m % 4] if m < 4 else kp[m - 4]
                            nc.vector.tensor_scalar(
                                out=dest[:, 512 * n2:512 * n2 + 512],
                                in0=self.pq[:, :],
                                scalar1=bqk_t[:, m:m + 1], scalar2=None,
                                op0=OP.add)
                        self.i += 1

            def vt_chunk(sc):
                """v^T for s-chunk sc, all heads: [128 s, 8*(64+1)] with a
                ones column per head (softmax denominator accumulator)."""
                pv = ap4.tile([128, 512], F32, tag="acc", name="acc")
                for kc in range(4):
                    nc.tensor.matmul(pv[:, :],
                                     xn[kc][:, 128 * sc:128 * sc + 128],
                                     wv[kc][:, :], start=(kc == 0), stop=(kc == 3))
                v3 = vt[sc][:, :].rearrange("p (h e) -> p h e", e=HD + 1)
                nc.vector.tensor_copy(vt[sc][:, HD::HD + 1], ones8_t[:, :])
                nc.vector.tensor_tensor(
                    out=v3[:, :, 0:HD],
                    in0=pv[:, :].rearrange("p (h e) -> p h e", e=HD),
                    in1=bvb_t[:, :].rearrange("p (h e) -> p h e", e=HD),
                    op=OP.add)

            def norm_head(p, e, n2, pa, act_copy=False):
                """softmax-normalize one AV accumulator into a_t.  The PSUM
                accumulator is drained immediately (denominator row + raw
                numerator) so its slot recycles fast; the normalization then
                runs SBUF-side in place.  act_copy routes the drain copies
                to ScalarE (for tail norms, when it has no exp work left)."""
                base = 64 * e
                asl = a_t[p][base:base + 64, 512 * n2:512 * n2 + 512]
                dsb = smallp.tile([1, 512], F32, tag="dsb", name="dsb")
                if act_copy:
                    nc.scalar.copy(dsb[:, :], pa[HD:HD + 1, :])
                    nc.scalar.copy(asl, pa[0:HD, :])
                else:
                    nc.vector.tensor_copy(dsb[:, :], pa[HD:HD + 1, :])
                    nc.vector.tensor_copy(asl, pa[0:HD, :])
                rcp = smallp.tile([1, 512], F32, tag="rcp", name="rcp")
                nc.vector.reciprocal_approx_fast(out=rcp[:, :],
                                                 in_=dsb[:, :])
                db = smallp.tile([128, 512], F32, tag="db", name="db")
                nc.gpsimd.partition_broadcast(db[:, :], rcp[:, :],
                                              channels=128)
                nc.vector.tensor_tensor(out=asl, in0=asl,
                                        in1=db[base:base + 64, :],
                                        op=OP.mult)

            class AvStream:
                """AV accumulation sweeps as an emit-on-demand stream
                (8 matmuls per sweep; norm emitted when a sweep closes).
                lockstep=True advances all sweeps one s-chunk at a time so a
                pair's own sweeps can ride its S loop, gated only on the
                exp tiles already produced."""
                def __init__(self, pe, sweeps, lockstep=False, act_copy=False,
                             pool=None):
                    self.p, self.est = pe
                    self.sweeps = sweeps
                    self.lockstep = lockstep
                    self.act_copy = act_copy
                    self.pool = pool or ap4
                    self.i = 0
                    self.pa = [None] * len(self.sweeps)

                def emit(self, k):
                    for _ in range(k):
                        if self.i >= 8 * len(self.sweeps):
                            return
                        if self.lockstep:
                            sweep = self.i % len(self.sweeps)
                            sc = self.i // len(self.sweeps)
                        else:
                            sweep, sc = divmod(self.i, 8)
                        e, n2 = self.sweeps[sweep]
                        h = 2 * self.p + e
                        if sc == 0:
                            self.pa[sweep] = self.pool.tile(
                                [HD + 1, 512], F32,
                                tag="acc" if self.pool is ap4 else "sps",
                                name="acc" if self.pool is ap4 else "sps")
                        nc.tensor.matmul(
                            self.pa[sweep][:, :], vt[sc][:, 65 * h:65 * h + 65],
                            self.est[e][sc][:, 512 * n2:512 * n2 + 512],
                            start=(sc == 0), stop=(sc == 7))
                        if sc == 7:
                            norm_head(self.p, e, n2, self.pa[sweep],
                                      act_copy=self.act_copy)
                        self.i += 1

            def s_mm(p, e, sc, est):
                """one head's S^T chunk + its exp"""
                base = 64 * e
                ps_s = spp.tile([128, L], F32, tag="sps", name="sps")
                for n2 in range(2):
                    nc.tensor.matmul(
                        ps_s[:, 512 * n2:512 * n2 + 512],
                        kp[p][base:base + 64, 128 * sc:128 * sc + 128],
                        qp[p][base:base + 64, 512 * n2:512 * n2 + 512],
                        start=True, stop=True, tile_position=(base, 0))
                es = expp.tile([128, L], BF16, tag="expS", name="expS")
                nc.scalar.activation(out=es[:, :], in_=ps_s[:, :], func=AF.Exp)
                est[e][sc] = es

            def attn_S(p, prev=None, qkv=None, own=None, stream_vt=False):
                """S^T + exp for pair p; the previous pair's AV sweeps, pair
                p+1's qkv, and (for p=3) the pair's own first sweep ride
                along ahead of the S matmuls.  The e=0/e=1 S matmul pairs
                stay adjacent so their disjoint row-groups execute
                concurrently on the PE; with 3 S-PSUM slots they are gated
                by the exp three allocations back, which has always
                drained."""
                est = [[None] * 8, [None] * 8]
                if own is not None:
                    own.est = est
                for sc in range(8):
                    s_mm(p, 0, sc, est)
                    s_mm(p, 1, sc, est)
                    if own is not None and sc >= 1:
                        own.emit(1)
                    if prev is not None:
                        prev.emit(4)
                    if stream_vt:
                        vt_chunk(sc)
                    if qkv is not None:
                        qkv.emit(2)
                return est

            # ---------- emission schedule ----------
            prev = None
            own3 = None
            for p in range(4):
                qs = QkvStream([p + 1, p + 5]) if p + 1 < 4 else None
                own3 = AvStream((p, None), sweeps=[(0, 0)], lockstep=True,
                                act_copy=True) if p == 3 else None
                est_cur = attn_S(p, prev, qs, own=own3, stream_vt=(p == 0))
                if prev is not None:
                    prev.emit(32)  # drain previous pair's sweeps
                if qs is not None:
                    qs.emit(16)    # drain qkv remainder
                prev = AvStream((p, est_cur),
                                sweeps=[(0, 0), (1, 0), (0, 1), (1, 1)])
            own3.emit(16)          # drain pair 3's first sweep + its norm
            # tail sweeps use the now-free S PSUM slots so all three can be
            # in flight at once instead of serializing through 2 slots
            av_rest = AvStream((3, prev.est),
                               sweeps=[(1, 0), (1, 1), (0, 1)], act_copy=True,
                               pool=spp)

            class ProjStream:
                """proj groups (m, n2): 4 accumulating matmuls then fused
                bias+residual and the output DMA.  body(m) emits the three
                cc<3 matmuls (gated only on pairs 0-2, long since ready);
                close(m) emits the cc=3 matmul (gated on pair 3's norms) and
                the drain.  The drain alternates DVE and ScalarE."""
                def __init__(self, n2):
                    self.n2 = n2
                    self.po = {}

                def body(self, m):
                    self.po[m] = spp.tile([128, 512], F32, tag="sps",
                                          name="sps")
                    for cc in range(3):
                        nc.tensor.matmul(self.po[m][:, :],
                                         wp[cc][:, 128 * m:128 * m + 128],
                                         a_t[cc][:, 512 * self.n2:512 * self.n2 + 512],
                                         start=(cc == 0), stop=False)

                def close(self, m, act=False):
                    n2 = self.n2
                    nc.tensor.matmul(self.po[m][:, :],
                                     wp[3][:, 128 * m:128 * m + 128],
                                     a_t[3][:, 512 * n2:512 * n2 + 512],
                                     start=False, stop=True)
                    ob = outp.tile([128, 512], F32, tag="ob", name="ob")
                    osl = out_d[128 * m:128 * m + 128, 512 * n2:512 * n2 + 512]
                    # ScalarE drains psum+bias (fast slot release, off the
                    # congested DVE); DVE then adds the residual in place
                    nc.scalar.activation(out=ob[:, :], in_=self.po[m][:, :],
                                         func=AF.Identity,
                                         bias=pb_t[:, m:m + 1])
                    nc.vector.tensor_tensor(
                        out=ob[:, :], in0=ob[:, :],
                        in1=xbt[m][:, 512 * n2:512 * n2 + 512], op=OP.add)
                    nc.sync.dma_start(out=osl, in_=ob[:, :])

            if dbg_d is not None:
                dbg_src = _CACHE.get("debug_src", "xn")
                src = {"xn": xn, "qp": qp, "kp": kp, "at": a_t}[dbg_src]
                for j in range(4):
                    dcp = outp.tile([128, L], F32, tag="dbg", name="dbg")
                    nc.vector.tensor_copy(dcp[:, :], src[j][:, :])
                    nc.sync.dma_start(out=dbg_d[128 * j:128 * j + 128, :],
                                      in_=dcp[:, :])

            # tail: pair 3's remaining sweeps interleaved with proj bodies
            # (only pairs 0-2 needed) so the PE never idles; group closes
            # follow as pair 3's norms land.  Allocation order matches the
            # release order of the 3 S-pool slots to avoid FIFO deadlock.
            pr0, pr1 = ProjStream(0), ProjStream(1)
            av_rest.emit(8)    # (1,0)
            av_rest.emit(8)    # (1,1)
            pr0.body(0)
            av_rest.emit(8)    # (0,1)
            pr0.body(1)
            pr0.close(0)
            pr0.body(2)
            pr0.close(1)
            pr0.body(3)
            pr0.close(2)
            pr0.close(3)
            pr1.body(0)
            pr1.body(1)
            pr1.body(2)
            pr1.close(0)
            pr1.body(3)
            pr1.close(1)
            pr1.close(2)
            pr1.close(3)

    nc.compile()
    _CACHE["nc"] = nc
    return nc


def _prep_constants(norm_w, norm_b, qkv_w, qkv_b, proj_w, proj_b):
    norm_w = np.asarray(norm_w, np.float64)
    norm_b = np.asarray(norm_b, np.float64)
    qkv_w = np.asarray(qkv_w, np.float64)
    qkv_b = np.asarray(qkv_b, np.float64)
    proj_w = np.asarray(proj_w, np.float64)
    proj_b = np.asarray(proj_b, np.float64)

    idx = np.arange(HD)
    q_idx = np.concatenate([h * 3 * HD + idx for h in range(H)])
    k_idx = q_idx + HD
    v_idx = q_idx + 2 * HD

    # fold norm affine: qkv = W @ (gn*nw + nb) + b = (W*nw) @ gn + (W@nb + b)
    Wf = qkv_w * norm_w[None, :]
    bf = qkv_b + qkv_w @ norm_b
    s2 = 1.0 / np.sqrt(HD)  # both q*scale and k*scale -> fold s^2 into q
    Wq, bq = Wf[q_idx] * s2, bf[q_idx] * s2
    Wk, bk = Wf[k_idx], bf[k_idx]
    Wv, bv = Wf[v_idx], bf[v_idx]

    wqk = np.concatenate([Wq.T, Wk.T], axis=1)                  # [512, 1024]
    bqk = np.concatenate([bq, bk]).reshape(8, 128).T            # [128, 8]
    wv = np.ascontiguousarray(Wv.T)                             # [512, 512]
    wp = np.ascontiguousarray(proj_w.T)                         # [512, 512]
    pb = proj_b.reshape(4, 128).T                               # [128, 4]

    # gfw column block j (used as lhsT [128, 32] for channel chunk j): maps
    # channel 128j+p to its global group 8j + p//16.
    gfw = np.zeros((128, 128), np.float64)
    for j in range(4):
        for p_ in range(128):
            gfw[p_, 32 * j + 8 * j + p_ // GSZ] = 1.0
    ch = np.arange(C)
    gbw = (ch[None, :] // GSZ == np.arange(G)[:, None]).astype(np.float64)

    import ml_dtypes
    f = np.float32
    bf16 = ml_dtypes.bfloat16
    return dict(ones8=np.ones((128, 8), bf16),
                wqk=np.ascontiguousarray(wqk.astype(bf16)),
                bqk=np.ascontiguousarray(bqk, f),
                wv=np.ascontiguousarray(wv.astype(bf16)),
                bvb=np.ascontiguousarray(bv[None, :], f),
                wp=np.ascontiguousarray(wp.astype(bf16)),
                pb=np.ascontiguousarray(pb, f),
                gfw=np.ascontiguousarray(gfw, f),
                gbw=np.ascontiguousarray(gbw, f))


def kernel(x, norm_w, norm_b, qkv_w, qkv_b, proj_w, proj_b, _trace=False):
    x = np.asarray(x, np.float32)
    consts = _prep_constants(norm_w, norm_b, qkv_w, qkv_b, proj_w, proj_b)
    nc = _build_module()
    in_maps = []
    import ml_dtypes as _md
    for i in range(N_CORES):
        xi = np.ascontiguousarray(x[i].reshape(C, L))
        m = {"xb": np.ascontiguousarray(xi.astype(_md.bfloat16))}
        m.update(consts)
        in_maps.append(m)
    res = run_bass_kernel_spmd(nc, in_maps, core_ids=list(range(N_CORES)),
                               trace=_trace)
    out = np.stack([res.results[i]["out"] for i in range(N_CORES)])
    if _trace:
        _CACHE["last_results"] = res
    return out.reshape(B, C, HH, WW).astype(np.float32)
